# revision 5
# baseline (speedup 1.0000x reference)
"""Trainium2 Bass kernel for nn_Attention_49082886259369.

Computes, per batch b (one batch per NeuronCore, 8 cores data-parallel):
    fac  = tanh(k @ W + q @ U)            [S, D]
    s    = v^T @ fac                      [D, D]
    attn = softmax(s, axis=batch)         <- couples cores
    out  = v @ attn                       [S, D]

v2 design (vs the 3-pass bf16-split v1):
  - All of mm1/mm2 run as single-pass float32r matmuls (PE truncates fp32
    operands to ~fp22, full bf16 rate at moving-dim >= 256). Emulating fp22
    truncation in numpy gives output rel err ~1.0e-3 against the fp64
    reference (gate 2e-2) -- the 3-pass split is unnecessary. PE work drops
    from 10 pass-units (547us floor) to 4 (219us floor).
  - mm2 accumulates over the S-contraction directly in PSUM (16 chained
    matmuls per (di, e-half)) instead of per-mi DVE adds: needs fac for all
    mi resident (64KB/partition) but removes ~137us of DVE work.
  - The dim-0 softmax is done with TWO AllToAlls instead of AR-max+AR-sum:
    each core takes ownership of a 128-wide e-slice, receives s (fp16) for
    all 8 batches, computes max/exp/sum/div locally in fp32, and sends
    attn (bf16) back. A2A moves ~2x fewer wire bytes than AllReduce and
    needs no second reduction. Chunked in 2 di-halves so chunk 0's
    collectives/softmax overlap chunk 1's mm2 and chunk 0's mm3 overlaps
    chunk 1's collectives.
  - mm3 runs in bf16 (attn quantized bf16 on the wire; benign -- attn in
    [0,1], output-proportional error).

Host-side layouts (all DMAs are [128 partitions x contiguous free]):
  kT/qT: [MT, 128, DT, 128] f32  [mi,p,di,sj] = x[mi*128+sj, di*128+p]
  W/U:   [128, DT, D]       f32  [p,di,e]     = W[di*128+p, e]
  vS:    [DT, 128, MT, 128] f32  [di,p,mi,j]  = v[mi*128+p, di*128+j]
  vT:    [MT, 128, DT, 128] bf16 [mi,p,di,tj] = v[mi*128+tj, di*128+p]
"""

import os

import numpy as np
import ml_dtypes

B, S, D = 8, 2048, 1024
NCORES = 8
P = 128
NE = 512  # matmul free-dim tile (one PSUM bank of fp32)
NCH = 2   # di-chunks for collective pipelining

_CACHE: dict = {}


# --------------------------------------------------------------------------
# device kernel builder
# --------------------------------------------------------------------------

def _build_nc(s_dim: int, d_dim: int, n_reps: int = 1, variant: str = "full"):
    import concourse.mybir as mybir
    import concourse.tile as tile
    from concourse import bacc

    F32 = mybir.dt.float32
    F32R = mybir.dt.float32r
    BF16 = mybir.dt.bfloat16
    F16 = mybir.dt.float16
    ACT = mybir.ActivationFunctionType

    MT = s_dim // P          # row tiles of S
    DT = d_dim // P          # row tiles of D
    DC = DT // NCH           # di per chunk
    EC = d_dim // NCORES     # e-slice owned per core in the softmax

    nc = bacc.Bacc("TRN2", target_bir_lowering=False, num_devices=NCORES)

    d_kT = nc.dram_tensor("kT", [MT, P, DT, P], F32R, kind="ExternalInput")
    d_qT = nc.dram_tensor("qT", [MT, P, DT, P], F32R, kind="ExternalInput")
    d_W = nc.dram_tensor("W", [P, DT, d_dim], F32R, kind="ExternalInput")
    d_U = nc.dram_tensor("U", [P, DT, d_dim], F32R, kind="ExternalInput")
    d_vS = nc.dram_tensor("vS", [DT, P, MT, P], F32R, kind="ExternalInput")
    d_vT = nc.dram_tensor("vT", [MT, P, DT, P], BF16, kind="ExternalInput")
    d_out = nc.dram_tensor("out", [s_dim, d_dim], F32, kind="ExternalOutput")

    grp = [list(range(NCORES))]

    def cc_a2a(cin, cout):
        if variant == "nocc":
            nc.gpsimd.dma_start(out=cout[:], in_=cin[:])
        else:
            nc.gpsimd.collective_compute(
                "AllToAll",
                mybir.AluOpType.bypass,
                replica_groups=grp,
                ins=[cin.opt()],
                outs=[cout.opt()],
            )

    with tile.TileContext(nc) as tc:
        with (
            tc.tile_pool(name="dram", bufs=2, space="DRAM") as dram_pool,
        ):
          for _rep in range(n_reps):
            a2a1_out_t = []
            with tc.tile_pool(name="facp", bufs=1) as fac_pool:
                fac = fac_pool.tile([P, MT, d_dim], F32R, tag="fac")

                # ---- sweep A: fac = tanh(k@W + q@U), single-pass f32r ----
                with (
                    tc.tile_pool(name="wu", bufs=1) as wu_pool,
                    tc.tile_pool(name="kq", bufs=3) as kq_pool,
                    tc.tile_pool(name="fps", bufs=4, space="PSUM") as fac_psum,
                ):
                    # small first k/q tiles ahead of the 4MB W/U loads
                    kt0 = kq_pool.tile([P, DT, P], F32R, tag="kt")
                    qt0 = kq_pool.tile([P, DT, P], F32R, tag="qt")
                    nc.sync.dma_start(out=kt0, in_=d_kT[0])
                    nc.sync.dma_start(out=qt0, in_=d_qT[0])
                    w_sb = wu_pool.tile([P, DT, d_dim], F32R, tag="w")
                    u_sb = wu_pool.tile([P, DT, d_dim], F32R, tag="u")
                    # split by di-half so mi=0's first matmuls start early
                    h = DT // 2
                    nc.sync.dma_start(out=w_sb[:, :h], in_=d_W[:, :h])
                    nc.sync.dma_start(out=u_sb[:, :h], in_=d_U[:, :h])
                    nc.sync.dma_start(out=w_sb[:, h:], in_=d_W[:, h:])
                    nc.sync.dma_start(out=u_sb[:, h:], in_=d_U[:, h:])
                    for mi in range(MT):
                        if mi == 0:
                            kt, qt = kt0, qt0
                        else:
                            kt = kq_pool.tile([P, DT, P], F32R, tag="kt")
                            qt = kq_pool.tile([P, DT, P], F32R, tag="qt")
                            nc.sync.dma_start(out=kt, in_=d_kT[mi])
                            nc.sync.dma_start(out=qt, in_=d_qT[mi])
                        fps = [fac_psum.tile([P, NE], F32, tag=f"fps{x}", name=f"fps{x}") for x in range(2)]
                        n_chain = 2 * DT
                        n = 0
                        for di in range(DT):
                            for src, wu in ((kt, w_sb), (qt, u_sb)):
                                for ni in range(2):
                                    nc.tensor.matmul(
                                        fps[ni],
                                        src[:, di, :],
                                        wu[:, di, ni * NE:(ni + 1) * NE],
                                        start=(n == 0),
                                        stop=(n == n_chain - 1),
                                    )
                                n += 1
                        for ni in range(2):
                            nc.scalar.activation(
                                fac[:, mi, ni * NE:(ni + 1) * NE], fps[ni],
                                ACT.Tanh)

                # ---- sweep B: s = v^T @ fac, PSUM-accumulated over mi ----
                with (
                    tc.tile_pool(name="s16p", bufs=1) as s16_pool,
                    tc.tile_pool(name="vs", bufs=2) as vs_pool,
                    tc.tile_pool(name="sps", bufs=4, space="PSUM") as s_psum,
                ):
                    s16 = s16_pool.tile([P, DT, d_dim], F16, tag="s16")
                    for ch in range(NCH):
                        for dl in range(DC):
                            di = ch * DC + dl
                            vs = vs_pool.tile([P, MT, P], F32R, tag="vs")
                            nc.sync.dma_start(out=vs, in_=d_vS[di])
                            sps = [s_psum.tile([P, NE], F32, tag=f"sps{x}", name=f"sps{x}") for x in range(2)]
                            for mi in range(MT):
                                for ni in range(2):
                                    nc.tensor.matmul(
                                        sps[ni],
                                        vs[:, mi, :],
                                        fac[:, mi, ni * NE:(ni + 1) * NE],
                                        start=(mi == 0),
                                        stop=(mi == MT - 1),
                                    )
                            for ni in range(2):
                                nc.vector.tensor_copy(
                                    s16[:, di, ni * NE:(ni + 1) * NE], sps[ni])
                        # chunk complete: stage fp16 s and kick its AllToAll
                        cin = dram_pool.tile([NCORES, P, DC, EC], F16,
                                             tag=f"a2a1_in{ch}",
                                             name=f"a2a1_in{ch}")
                        cout = dram_pool.tile([NCORES, P, DC, EC], F16,
                                              tag=f"a2a1_out{ch}",
                                              name=f"a2a1_out{ch}")
                        dsl = slice(ch * DC, (ch + 1) * DC)
                        for c in range(NCORES):
                            nc.sync.dma_start(
                                out=cin[c],
                                in_=s16[:, dsl, c * EC:(c + 1) * EC])
                        cc_a2a(cin, cout)
                        a2a1_out_t.append(cout)

            # ---- softmax on the owned e-slice + A2A back, per chunk ----
            a2a2_out_t = []
            with tc.tile_pool(name="smp", bufs=1) as sm_pool:
                for ch in range(NCH):
                    r16 = sm_pool.tile([P, NCORES, DC, EC], F16, tag="r16",
                                       name=f"r16_{ch}")
                    for b in range(NCORES):
                        nc.sync.dma_start(out=r16[:, b],
                                          in_=a2a1_out_t[ch][b])
                    m4 = sm_pool.tile([P, 4, DC, EC], F16, tag="m4")
                    m2 = sm_pool.tile([P, 2, DC, EC], F16, tag="m2")
                    mf = sm_pool.tile([P, DC, EC], F16, tag="mf")
                    for i in range(4):
                        nc.vector.tensor_max(m4[:, i], r16[:, 2 * i],
                                             r16[:, 2 * i + 1])
                    for i in range(2):
                        nc.vector.tensor_max(m2[:, i], m4[:, 2 * i],
                                             m4[:, 2 * i + 1])
                    nc.vector.tensor_max(mf, m2[:, 0], m2[:, 1])
                    e16 = sm_pool.tile([P, NCORES, DC, EC], F16, tag="e16",
                                       name=f"e16_{ch}")
                    for b in range(NCORES):
                        nc.vector.tensor_sub(e16[:, b], r16[:, b], mf)
                    nc.scalar.activation(e16, e16, ACT.Exp)
                    a4 = sm_pool.tile([P, 4, DC, EC], F32, tag="a4")
                    a2 = sm_pool.tile([P, 2, DC, EC], F32, tag="a2")
                    den = sm_pool.tile([P, DC, EC], F32, tag="den")
                    for i in range(4):
                        nc.vector.tensor_add(a4[:, i], e16[:, 2 * i],
                                             e16[:, 2 * i + 1])
                    for i in range(2):
                        nc.vector.tensor_add(a2[:, i], a4[:, 2 * i],
                                             a4[:, 2 * i + 1])
                    nc.vector.tensor_add(den, a2[:, 0], a2[:, 1])
                    nc.vector.reciprocal(den, den)
                    at16 = sm_pool.tile([P, NCORES, DC, EC], BF16, tag="at16",
                                        name=f"at16_{ch}")
                    for b in range(NCORES):
                        nc.vector.tensor_mul(at16[:, b], e16[:, b], den)
                    cin = dram_pool.tile([NCORES, P, DC, EC], BF16,
                                         tag=f"a2a2_in{ch}",
                                         name=f"a2a2_in{ch}")
                    cout = dram_pool.tile([NCORES, P, DC, EC], BF16,
                                          tag=f"a2a2_out{ch}",
                                          name=f"a2a2_out{ch}")
                    for b in range(NCORES):
                        nc.sync.dma_start(out=cin[b], in_=at16[:, b])
                    cc_a2a(cin, cout)
                    a2a2_out_t.append(cout)

            # ---- mm3: out = v @ attn (bf16), chunked over di-halves ----
            with (
                tc.tile_pool(name="attnp", bufs=1) as attn_pool,
                tc.tile_pool(name="ostp", bufs=1) as ost_pool,
                tc.tile_pool(name="vt", bufs=3) as vt_pool,
                tc.tile_pool(name="ops", bufs=4, space="PSUM") as out_psum,
            ):
                attn_sb = attn_pool.tile([P, DT, d_dim], BF16, tag="attn")
                ost = [
                    ost_pool.tile([P, d_dim], F32, tag=f"ost{mi}",
                                  name=f"ost{mi}")
                    for mi in range(MT)
                ]
                for ch in range(NCH):
                    dsl = slice(ch * DC, (ch + 1) * DC)
                    for c in range(NCORES):
                        nc.sync.dma_start(
                            out=attn_sb[:, dsl, c * EC:(c + 1) * EC],
                            in_=a2a2_out_t[ch][c])
                    for mi in range(MT):
                        vt = vt_pool.tile([P, DC, P], BF16, tag="vt")
                        nc.sync.dma_start(out=vt, in_=d_vT[mi][:, dsl, :])
                        ops = [out_psum.tile([P, NE], F32, tag=f"ops{x}", name=f"ops{x}") for x in range(2)]
                        for dl in range(DC):
                            di = ch * DC + dl
                            for ni in range(2):
                                nc.tensor.matmul(
                                    ops[ni],
                                    vt[:, dl, :],
                                    attn_sb[:, di, ni * NE:(ni + 1) * NE],
                                    start=(dl == 0),
                                    stop=(dl == DC - 1),
                                )
                        for ni in range(2):
                            esl = slice(ni * NE, (ni + 1) * NE)
                            if ch == 0:
                                nc.scalar.activation(ost[mi][:, esl], ops[ni],
                                                     ACT.Copy)
                            else:
                                nc.vector.tensor_add(ost[mi][:, esl],
                                                     ost[mi][:, esl], ops[ni])
                        if ch == NCH - 1:
                            nc.sync.dma_start(
                                out=d_out[mi * P:(mi + 1) * P, :],
                                in_=ost[mi])
            tc.tile_update_base_wait()

    nc.compile()
    return nc


def _get_nc(s_dim=S, d_dim=D, n_reps=1, variant="full"):
    key = ("nc", s_dim, d_dim, n_reps, variant)
    if key not in _CACHE:
        _CACHE[key] = _build_nc(s_dim, d_dim, n_reps, variant)
    return _CACHE[key]


# --------------------------------------------------------------------------
# host-side packing
# --------------------------------------------------------------------------

def _tileT(x: np.ndarray, s_dim: int, d_dim: int) -> np.ndarray:
    """[S, D] -> [MT, 128, DT, 128] with [mi,p,di,sj] = x[mi*128+sj, di*128+p]."""
    mt, dt = s_dim // P, d_dim // P
    return np.ascontiguousarray(
        x.reshape(mt, P, dt, P).transpose(0, 3, 2, 1)
    )


def prepare_in_maps(q, k, v, W, U, s_dim=S, d_dim=D):
    q = np.asarray(q, dtype=np.float32)
    k = np.asarray(k, dtype=np.float32)
    v = np.asarray(v, dtype=np.float32)
    W = np.asarray(W, dtype=np.float32)
    U = np.asarray(U, dtype=np.float32)

    dt = d_dim // P
    mt = s_dim // P
    W_t = np.ascontiguousarray(W.reshape(dt, P, d_dim).transpose(1, 0, 2))
    U_t = np.ascontiguousarray(U.reshape(dt, P, d_dim).transpose(1, 0, 2))

    in_maps = []
    for b in range(NCORES):
        kT = _tileT(k[b], s_dim, d_dim)
        qT = _tileT(q[b], s_dim, d_dim)
        vT = _tileT(v[b], s_dim, d_dim).astype(ml_dtypes.bfloat16)
        vS = np.ascontiguousarray(
            v[b].reshape(mt, P, dt, P).transpose(2, 1, 0, 3))
        in_maps.append({
            "kT": kT, "qT": qT,
            "W": W_t, "U": U_t,
            "vS": vS, "vT": vT,
        })
    return in_maps


def run_spmd(in_maps, s_dim=S, d_dim=D):
    """One-shot path through the stock bass_utils helper (debug use)."""
    from concourse import bass_utils
    nc = _get_nc(s_dim, d_dim)
    res = bass_utils.run_bass_kernel_spmd(
        nc, in_maps=in_maps, core_ids=list(range(NCORES))
    )
    return res


def _get_runner(s_dim=S, d_dim=D, n_reps=1, variant="full"):
    """Cached sharded-jit runner over the same bass2jax/_bass_exec_p path
    that bass_utils.run_bass_kernel_spmd uses under axon, but built once per
    process (no donation) so repeat calls skip re-trace/re-compile."""
    key = ("runner", s_dim, d_dim, n_reps, variant)
    if key in _CACHE:
        return _CACHE[key]

    import jax
    from jax.sharding import Mesh, PartitionSpec
    from jax.experimental.shard_map import shard_map
    import concourse.mybir as mybir
    from concourse import bass2jax

    nc = _get_nc(s_dim, d_dim, n_reps, variant)
    bass2jax.install_neuronx_cc_hook()

    partition_name = (
        nc.partition_id_tensor.name if nc.partition_id_tensor else None
    )
    in_names, out_names, out_avals, zero_outs = [], [], [], []
    for alloc in nc.m.functions[0].allocations:
        if not isinstance(alloc, mybir.MemoryLocationSet):
            continue
        name = alloc.memorylocations[0].name
        if alloc.kind == "ExternalInput":
            if name != partition_name:
                in_names.append(name)
        elif alloc.kind == "ExternalOutput":
            shape = tuple(alloc.tensor_shape)
            dtype = mybir.dt.np(alloc.dtype)
            out_names.append(name)
            out_avals.append(jax.core.ShapedArray(shape, dtype))
            zero_outs.append(np.zeros(shape, dtype))
    n_params = len(in_names)
    all_in_names = list(in_names) + list(out_names)
    if partition_name is not None:
        all_in_names.append(partition_name)

    def _body(*args):
        operands = list(args)
        if partition_name is not None:
            operands.append(bass2jax.partition_id_tensor())
        outs = bass2jax._bass_exec_p.bind(
            *operands,
            out_avals=tuple(out_avals),
            in_names=tuple(all_in_names),
            out_names=tuple(out_names),
            lowering_input_output_aliases=(),
            sim_require_finite=True,
            sim_require_nnan=True,
            nc=nc,
        )
        return tuple(outs)

    devices = jax.devices()[:NCORES]
    mesh = Mesh(np.asarray(devices), ("core",))
    in_specs = (PartitionSpec("core"),) * (n_params + len(out_names))
    out_specs = (PartitionSpec("core"),) * len(out_names)
    sharded = jax.jit(
        shard_map(
            _body, mesh=mesh, in_specs=in_specs, out_specs=out_specs,
            check_rep=False,
        ),
        keep_unused=True,
    )
    runner = {
        "fn": sharded,
        "in_names": in_names,
        "out_names": out_names,
        "out_avals": out_avals,
        "zero_concat": [
            np.zeros((NCORES * z.shape[0], *z.shape[1:]), z.dtype)
            for z in zero_outs
        ],
        "mesh": mesh,
    }
    _CACHE[key] = runner
    return runner


def _concat_inputs(runner, in_maps):
    return [
        np.concatenate([np.asarray(m[name]) for m in in_maps], axis=0)
        for name in runner["in_names"]
    ]


def run_fast(in_maps, s_dim=S, d_dim=D):
    """Execute via the cached runner; returns list of per-core out dicts."""
    runner = _get_runner(s_dim, d_dim)
    concat_in = _concat_inputs(runner, in_maps)
    out_arrs = runner["fn"](*concat_in, *runner["zero_concat"])
    results = []
    for c in range(NCORES):
        results.append({
            name: np.asarray(out_arrs[i]).reshape(
                NCORES, *runner["out_avals"][i].shape
            )[c]
            for i, name in enumerate(runner["out_names"])
        })
    return results


def timed_run(in_maps, iters=20, s_dim=S, d_dim=D, n_reps=1, variant="full"):
    """Steady-state timing with device-resident inputs. Returns (min_s, all)."""
    import time
    import jax
    from jax.sharding import NamedSharding, PartitionSpec

    runner = _get_runner(s_dim, d_dim, n_reps, variant)
    sh = NamedSharding(runner["mesh"], PartitionSpec("core"))
    dev_in = [jax.device_put(a, sh) for a in _concat_inputs(runner, in_maps)]
    dev_zero = [jax.device_put(z, sh) for z in runner["zero_concat"]]
    jax.block_until_ready(dev_in)
    jax.block_until_ready(dev_zero)
    # warmup (also triggers compile on first use)
    jax.block_until_ready(runner["fn"](*dev_in, *dev_zero))
    times = []
    for _ in range(iters):
        t0 = time.perf_counter()
        jax.block_until_ready(runner["fn"](*dev_in, *dev_zero))
        times.append(time.perf_counter() - t0)
    return min(times), times


def kernel(q, k, v, W, U):
    in_maps = prepare_in_maps(q, k, v, W, U)
    if os.environ.get("BASS_USE_SPMD_HELPER"):
        res = run_spmd(in_maps)
        results = res.results
    else:
        results = run_fast(in_maps)
    out = np.stack([results[b]["out"] for b in range(NCORES)], axis=0)
    return out.astype(np.float32)


def timed_slope(in_maps, iters=30, reps_hi=3, s_dim=S, d_dim=D, variant="full"):
    """True HW kernel time via replication slope: the reps_hi variant runs
    the whole kernel body reps_hi times inside one NEFF. Calls of the two
    variants are interleaved in one loop so slow network drift cancels;
    returns (per_rep_seconds from median pairwise delta, t1_min, thi_min)."""
    import time
    import jax
    from jax.sharding import NamedSharding, PartitionSpec

    runners = {}
    for n in (1, reps_hi):
        r = _get_runner(s_dim, d_dim, n, variant)
        sh = NamedSharding(r["mesh"], PartitionSpec("core"))
        dev_in = [jax.device_put(a, sh) for a in _concat_inputs(r, in_maps)]
        dev_zero = [jax.device_put(z, sh) for z in r["zero_concat"]]
        jax.block_until_ready(dev_in)
        jax.block_until_ready(dev_zero)
        jax.block_until_ready(r["fn"](*dev_in, *dev_zero))  # warm/compile
        runners[n] = (r["fn"], dev_in, dev_zero)

    deltas, t1s, this_ = [], [], []
    for _ in range(iters):
        fn, di, dz = runners[1]
        t0 = time.perf_counter()
        jax.block_until_ready(fn(*di, *dz))
        t1 = time.perf_counter() - t0
        fn, di, dz = runners[reps_hi]
        t0 = time.perf_counter()
        jax.block_until_ready(fn(*di, *dz))
        th = time.perf_counter() - t0
        deltas.append(th - t1)
        t1s.append(t1)
        this_.append(th)
    deltas.sort()
    med = deltas[len(deltas) // 2]
    return med / (reps_hi - 1), min(t1s), min(this_)


# revision 12
# speedup vs baseline: 1.1291x; 1.1291x over previous
"""Trainium2 Bass kernel for nn_Attention_49082886259369.

Computes, per batch b (one batch per NeuronCore, 8 cores data-parallel):
    fac  = tanh(k @ W + q @ U)            [S, D]
    s    = v^T @ fac                      [D, D]
    attn = softmax(s, axis=batch)         <- couples cores
    out  = v @ attn                       [S, D]

v2 design (vs the 3-pass bf16-split v1):
  - All of mm1/mm2 run as single-pass float32r matmuls (PE truncates fp32
    operands to ~fp22, full bf16 rate at moving-dim >= 256). Emulating fp22
    truncation in numpy gives output rel err ~1.0e-3 against the fp64
    reference (gate 2e-2) -- the 3-pass split is unnecessary. PE work drops
    from 10 pass-units (547us floor) to 4 (219us floor).
  - mm2 accumulates over the S-contraction directly in PSUM (16 chained
    matmuls per (di, e-half)) instead of per-mi DVE adds: needs fac for all
    mi resident (64KB/partition) but removes ~137us of DVE work.
  - The dim-0 softmax is done with TWO AllToAlls instead of AR-max+AR-sum:
    each core takes ownership of a 128-wide e-slice, receives s (fp16) for
    all 8 batches, computes max/exp/sum/div locally in fp32, and sends
    attn (bf16) back. A2A moves ~2x fewer wire bytes than AllReduce and
    needs no second reduction. Chunked in 2 di-halves so chunk 0's
    collectives/softmax overlap chunk 1's mm2 and chunk 0's mm3 overlaps
    chunk 1's collectives.
  - mm3 runs in bf16 (attn quantized bf16 on the wire; benign -- attn in
    [0,1], output-proportional error).

Host-side layouts (all DMAs are [128 partitions x contiguous free]):
  kT/qT: [MT, 128, DT, 128] f32  [mi,p,di,sj] = x[mi*128+sj, di*128+p]
  W/U:   [128, DT, D]       f32  [p,di,e]     = W[di*128+p, e]
  vS:    [DT, 128, MT, 128] f32  [di,p,mi,j]  = v[mi*128+p, di*128+j]
  vT:    [MT, 128, DT, 128] bf16 [mi,p,di,tj] = v[mi*128+tj, di*128+p]
"""

import os

import numpy as np
import ml_dtypes

B, S, D = 8, 2048, 1024
NCORES = 8
P = 128
NE = 512  # matmul free-dim tile (one PSUM bank of fp32)
NCH = 2   # di-chunks for collective pipelining

_CACHE: dict = {}


# --------------------------------------------------------------------------
# device kernel builder
# --------------------------------------------------------------------------

def _build_nc(s_dim: int, d_dim: int, n_reps: int = 1, variant: str = "full"):
    import concourse.mybir as mybir
    import concourse.tile as tile
    from concourse import bacc

    F32 = mybir.dt.float32
    F32R = mybir.dt.float32r
    BF16 = mybir.dt.bfloat16
    F16 = mybir.dt.float16
    ACT = mybir.ActivationFunctionType

    MT = s_dim // P          # row tiles of S
    DT = d_dim // P          # row tiles of D
    DC = DT // NCH           # di per chunk
    EC = d_dim // NCORES     # e-slice owned per core in the softmax

    nc = bacc.Bacc("TRN2", target_bir_lowering=False, num_devices=NCORES)

    d_kT = nc.dram_tensor("kT", [MT, P, DT, P], F32R, kind="ExternalInput")
    d_qT = nc.dram_tensor("qT", [MT, P, DT, P], F32R, kind="ExternalInput")
    d_W = nc.dram_tensor("W", [P, DT, d_dim], F32R, kind="ExternalInput")
    d_U = nc.dram_tensor("U", [P, DT, d_dim], F32R, kind="ExternalInput")
    d_vS = nc.dram_tensor("vS", [DT, P, MT, P], F32R, kind="ExternalInput")
    d_vT = nc.dram_tensor("vT", [MT, P, DT, P], BF16, kind="ExternalInput")
    d_out = nc.dram_tensor("out", [s_dim, d_dim], F32, kind="ExternalOutput")

    grp = [list(range(NCORES))]

    def cc_a2a(cin, cout):
        if variant == "nocc":
            nc.gpsimd.dma_start(out=cout[:], in_=cin[:])
        else:
            nc.gpsimd.collective_compute(
                "AllToAll",
                mybir.AluOpType.bypass,
                replica_groups=grp,
                ins=[cin.opt()],
                outs=[cout.opt()],
            )

    with tile.TileContext(nc) as tc:
        with (
            tc.tile_pool(name="dram", bufs=2, space="DRAM") as dram_pool,
        ):
          for _rep in range(n_reps):
            a2a1_out_t = []
            with (
                tc.tile_pool(name="facp", bufs=1) as fac_pool,
                tc.tile_pool(name="s16p", bufs=1) as s16_pool,
                tc.tile_pool(name="vs", bufs=3) as vs_pool,
            ):
                fac = fac_pool.tile([P, MT, d_dim], F32R, tag="fac")
                s16 = s16_pool.tile([P, DT, d_dim], F16, tag="s16")
                # prefetch the first v slices now: their pool doesn't overlap
                # the sweep-A pools, so these DMAs run during sweep A and
                # sweep B can start the moment fac completes
                vs_tiles = {}
                for di in range(2):
                    vs = vs_pool.tile([P, MT, P], F32R, tag="vs",
                                      name=f"vs_pre{di}")
                    nc.sync.dma_start(out=vs, in_=d_vS[di])
                    vs_tiles[di] = vs

                # ---- sweep A: fac = tanh(k@W + q@U), single-pass f32r ----
                with (
                    tc.tile_pool(name="wu", bufs=1) as wu_pool,
                    tc.tile_pool(name="kq", bufs=3) as kq_pool,
                    tc.tile_pool(name="fps", bufs=4, space="PSUM") as fac_psum,
                ):
                    # small first k/q tiles ahead of the 4MB W/U loads
                    kt0 = kq_pool.tile([P, DT, P], F32R, tag="kt")
                    qt0 = kq_pool.tile([P, DT, P], F32R, tag="qt")
                    nc.sync.dma_start(out=kt0, in_=d_kT[0])
                    nc.sync.dma_start(out=qt0, in_=d_qT[0])
                    w_sb = wu_pool.tile([P, DT, d_dim], F32R, tag="w")
                    u_sb = wu_pool.tile([P, DT, d_dim], F32R, tag="u")
                    # per-di interleaved loads: mi=0's chain starts after just
                    # W[0]/U[0] land and never stalls long enough to lose the
                    # PE clock ramp
                    for di in range(DT):
                        nc.sync.dma_start(out=w_sb[:, di], in_=d_W[:, di])
                        nc.sync.dma_start(out=u_sb[:, di], in_=d_U[:, di])
                    for mi in range(MT):
                        if mi == 0:
                            kt, qt = kt0, qt0
                        else:
                            kt = kq_pool.tile([P, DT, P], F32R, tag="kt")
                            qt = kq_pool.tile([P, DT, P], F32R, tag="qt")
                            nc.sync.dma_start(out=kt, in_=d_kT[mi])
                            nc.sync.dma_start(out=qt, in_=d_qT[mi])
                        fps = [fac_psum.tile([P, NE], F32, tag=f"fps{x}", name=f"fps{x}") for x in range(2)]
                        n_chain = 2 * DT
                        n = 0
                        for di in range(DT):
                            for src, wu in ((kt, w_sb), (qt, u_sb)):
                                for ni in range(2):
                                    nc.tensor.matmul(
                                        fps[ni],
                                        src[:, di, :],
                                        wu[:, di, ni * NE:(ni + 1) * NE],
                                        start=(n == 0),
                                        stop=(n == n_chain - 1),
                                    )
                                n += 1
                        for ni in range(2):
                            nc.scalar.activation(
                                fac[:, mi, ni * NE:(ni + 1) * NE], fps[ni],
                                ACT.Tanh)

                # ---- sweep B: s = v^T @ fac, PSUM-accumulated over mi ----
                with (
                    tc.tile_pool(name="sps", bufs=4, space="PSUM") as s_psum,
                ):
                    cins = [
                        dram_pool.tile([NCORES, P, DC, EC], F16,
                                       tag=f"a2a1_in{ch}", name=f"a2a1_in{ch}")
                        for ch in range(NCH)
                    ]
                    couts = [
                        dram_pool.tile([NCORES, P, DC, EC], F16,
                                       tag=f"a2a1_out{ch}",
                                       name=f"a2a1_out{ch}")
                        for ch in range(NCH)
                    ]
                    for ch in range(NCH):
                        for dl in range(DC):
                            di = ch * DC + dl
                            if di in vs_tiles:
                                vs = vs_tiles.pop(di)
                            else:
                                vs = vs_pool.tile([P, MT, P], F32R, tag="vs",
                                                  name=f"vs{di}")
                                nc.sync.dma_start(out=vs, in_=d_vS[di])
                            sps = [s_psum.tile([P, NE], F32, tag=f"sps{x}", name=f"sps{x}") for x in range(2)]
                            for mi in range(MT):
                                for ni in range(2):
                                    nc.tensor.matmul(
                                        sps[ni],
                                        vs[:, mi, :],
                                        fac[:, mi, ni * NE:(ni + 1) * NE],
                                        start=(mi == 0),
                                        stop=(mi == MT - 1),
                                    )
                            for ni in range(2):
                                nc.vector.tensor_copy(
                                    s16[:, di, ni * NE:(ni + 1) * NE], sps[ni])
                            # stage this di's fp16 s rows immediately so the
                            # chunk's AllToAll is gated only on the last di;
                            # transposed DRAM AP -> one DMA instead of 8
                            nc.scalar.dma_start(
                                out=cins[ch][:, :, dl, :].transpose([1, 0, 2]),
                                in_=s16[:, di, :].rearrange(
                                    "p (c e) -> p c e", c=NCORES))
                        cc_a2a(cins[ch], couts[ch])
                        a2a1_out_t.append(couts[ch])

            # ---- prefetch mm3's v tiles during the collective window ----
            vt_pool_cm = tc.tile_pool(name="vt", bufs=1)
            vt_pool = vt_pool_cm.__enter__()
            vt_all = []
            for ch in range(NCH):
                vt = vt_pool.tile([P, MT, DC, P], BF16, tag=f"vtall{ch}",
                                  name=f"vtall{ch}")
                nc.scalar.dma_start(
                    out=vt,
                    in_=d_vT[:, :, ch * DC:(ch + 1) * DC, :].transpose(
                        [1, 0, 2, 3]))
                vt_all.append(vt)

            # ---- softmax on the owned e-slice + A2A back, per chunk ----
            a2a2_out_t = []
            with tc.tile_pool(name="smp", bufs=1) as sm_pool:
                for ch in range(NCH):
                    r16 = sm_pool.tile([P, NCORES, DC, EC], F16, tag="r16",
                                       name=f"r16_{ch}")
                    nc.sync.dma_start(
                        out=r16,
                        in_=a2a1_out_t[ch][:].transpose([1, 0, 2, 3]))
                    m4 = sm_pool.tile([P, 4, DC, EC], F16, tag="m4")
                    m2 = sm_pool.tile([P, 2, DC, EC], F16, tag="m2")
                    mf = sm_pool.tile([P, DC, EC], F16, tag="mf")
                    for i in range(4):
                        nc.vector.tensor_max(m4[:, i], r16[:, 2 * i],
                                             r16[:, 2 * i + 1])
                    for i in range(2):
                        nc.vector.tensor_max(m2[:, i], m4[:, 2 * i],
                                             m4[:, 2 * i + 1])
                    nc.vector.tensor_max(mf, m2[:, 0], m2[:, 1])
                    e16 = sm_pool.tile([P, NCORES, DC, EC], F16, tag="e16",
                                       name=f"e16_{ch}")
                    for b in range(NCORES):
                        nc.vector.tensor_sub(e16[:, b], r16[:, b], mf)
                    nc.scalar.activation(e16, e16, ACT.Exp)
                    a4 = sm_pool.tile([P, 4, DC, EC], F32, tag="a4")
                    a2 = sm_pool.tile([P, 2, DC, EC], F32, tag="a2")
                    den = sm_pool.tile([P, DC, EC], F32, tag="den")
                    for i in range(4):
                        nc.vector.tensor_add(a4[:, i], e16[:, 2 * i],
                                             e16[:, 2 * i + 1])
                    for i in range(2):
                        nc.vector.tensor_add(a2[:, i], a4[:, 2 * i],
                                             a4[:, 2 * i + 1])
                    nc.vector.tensor_add(den, a2[:, 0], a2[:, 1])
                    nc.vector.reciprocal(den, den)
                    at16 = sm_pool.tile([P, NCORES, DC, EC], BF16, tag="at16",
                                        name=f"at16_{ch}")
                    for b in range(NCORES):
                        nc.vector.tensor_mul(at16[:, b], e16[:, b], den)
                    cin = dram_pool.tile([NCORES, P, DC, EC], BF16,
                                         tag=f"a2a2_in{ch}",
                                         name=f"a2a2_in{ch}")
                    cout = dram_pool.tile([NCORES, P, DC, EC], BF16,
                                          tag=f"a2a2_out{ch}",
                                          name=f"a2a2_out{ch}")
                    nc.scalar.dma_start(
                        out=cin[:].transpose([1, 0, 2, 3]), in_=at16)
                    cc_a2a(cin, cout)
                    a2a2_out_t.append(cout)

            # ---- mm3: out = v @ attn (bf16), chunked over di-halves ----
            with (
                tc.tile_pool(name="attnp", bufs=1) as attn_pool,
                tc.tile_pool(name="ostp", bufs=1) as ost_pool,
                tc.tile_pool(name="ops", bufs=3, space="PSUM") as out_psum,
                tc.tile_pool(name="wrm", bufs=1, space="PSUM") as warm_psum,
            ):
                attn_sb = attn_pool.tile([P, DT, d_dim], BF16, tag="attn")
                ost = [
                    ost_pool.tile([P, d_dim], F32, tag=f"ost{mi}",
                                  name=f"ost{mi}")
                    for mi in range(MT)
                ]
                for ch in range(NCH):
                    dsl = slice(ch * DC, (ch + 1) * DC)
                    if ch > 0:
                        # keep the PE clock ramp alive between mm3 chunks:
                        # ~5us of chained throwaway matmuls bridge the gap
                        # until the last AllToAll lands (HAM re-throttles
                        # after ~3.4us idle)
                        wps = warm_psum.tile([P, NE], F32, tag="warm",
                                             name=f"warm{ch}")
                        for wi in range(24):
                            nc.tensor.matmul(
                                ops[0] if False else wps,
                                vt_all[0][:, 0, 0, :],
                                attn_sb[:, 0, 0:NE],
                                start=(wi == 0),
                                stop=(wi == 23),
                            )
                    nc.sync.dma_start(
                        out=attn_sb[:, dsl, :].rearrange(
                            "p d (c e) -> p d c e", c=NCORES),
                        in_=a2a2_out_t[ch][:].transpose([1, 2, 0, 3]))
                    for mi in range(MT):
                        vt = vt_all[ch]
                        ops = [out_psum.tile([P, NE], F32, tag=f"ops{x}", name=f"ops{x}") for x in range(2)]
                        for dl in range(DC):
                            di = ch * DC + dl
                            for ni in range(2):
                                nc.tensor.matmul(
                                    ops[ni],
                                    vt[:, mi, dl, :],
                                    attn_sb[:, di, ni * NE:(ni + 1) * NE],
                                    start=(dl == 0),
                                    stop=(dl == DC - 1),
                                )
                        for ni in range(2):
                            esl = slice(ni * NE, (ni + 1) * NE)
                            if ch == 0:
                                nc.scalar.activation(ost[mi][:, esl], ops[ni],
                                                     ACT.Copy)
                            else:
                                nc.vector.tensor_add(ost[mi][:, esl],
                                                     ost[mi][:, esl], ops[ni])
                        if ch == NCH - 1:
                            nc.sync.dma_start(
                                out=d_out[mi * P:(mi + 1) * P, :],
                                in_=ost[mi])
            vt_pool_cm.__exit__(None, None, None)
            tc.tile_update_base_wait()

    nc.compile()
    return nc


def _get_nc(s_dim=S, d_dim=D, n_reps=1, variant="full"):
    key = ("nc", s_dim, d_dim, n_reps, variant)
    if key not in _CACHE:
        _CACHE[key] = _build_nc(s_dim, d_dim, n_reps, variant)
    return _CACHE[key]


# --------------------------------------------------------------------------
# host-side packing
# --------------------------------------------------------------------------

def _tileT(x: np.ndarray, s_dim: int, d_dim: int) -> np.ndarray:
    """[S, D] -> [MT, 128, DT, 128] with [mi,p,di,sj] = x[mi*128+sj, di*128+p]."""
    mt, dt = s_dim // P, d_dim // P
    return np.ascontiguousarray(
        x.reshape(mt, P, dt, P).transpose(0, 3, 2, 1)
    )


def prepare_in_maps(q, k, v, W, U, s_dim=S, d_dim=D):
    q = np.asarray(q, dtype=np.float32)
    k = np.asarray(k, dtype=np.float32)
    v = np.asarray(v, dtype=np.float32)
    W = np.asarray(W, dtype=np.float32)
    U = np.asarray(U, dtype=np.float32)

    dt = d_dim // P
    mt = s_dim // P
    W_t = np.ascontiguousarray(W.reshape(dt, P, d_dim).transpose(1, 0, 2))
    U_t = np.ascontiguousarray(U.reshape(dt, P, d_dim).transpose(1, 0, 2))

    in_maps = []
    for b in range(NCORES):
        kT = _tileT(k[b], s_dim, d_dim)
        qT = _tileT(q[b], s_dim, d_dim)
        vT = _tileT(v[b], s_dim, d_dim).astype(ml_dtypes.bfloat16)
        vS = np.ascontiguousarray(
            v[b].reshape(mt, P, dt, P).transpose(2, 1, 0, 3))
        in_maps.append({
            "kT": kT, "qT": qT,
            "W": W_t, "U": U_t,
            "vS": vS, "vT": vT,
        })
    return in_maps


def run_spmd(in_maps, s_dim=S, d_dim=D):
    """One-shot path through the stock bass_utils helper (debug use)."""
    from concourse import bass_utils
    nc = _get_nc(s_dim, d_dim)
    res = bass_utils.run_bass_kernel_spmd(
        nc, in_maps=in_maps, core_ids=list(range(NCORES))
    )
    return res


def _get_runner(s_dim=S, d_dim=D, n_reps=1, variant="full"):
    """Cached sharded-jit runner over the same bass2jax/_bass_exec_p path
    that bass_utils.run_bass_kernel_spmd uses under axon, but built once per
    process (no donation) so repeat calls skip re-trace/re-compile."""
    key = ("runner", s_dim, d_dim, n_reps, variant)
    if key in _CACHE:
        return _CACHE[key]

    import jax
    from jax.sharding import Mesh, PartitionSpec
    from jax.experimental.shard_map import shard_map
    import concourse.mybir as mybir
    from concourse import bass2jax

    nc = _get_nc(s_dim, d_dim, n_reps, variant)
    bass2jax.install_neuronx_cc_hook()

    partition_name = (
        nc.partition_id_tensor.name if nc.partition_id_tensor else None
    )
    in_names, out_names, out_avals, zero_outs = [], [], [], []
    for alloc in nc.m.functions[0].allocations:
        if not isinstance(alloc, mybir.MemoryLocationSet):
            continue
        name = alloc.memorylocations[0].name
        if alloc.kind == "ExternalInput":
            if name != partition_name:
                in_names.append(name)
        elif alloc.kind == "ExternalOutput":
            shape = tuple(alloc.tensor_shape)
            dtype = mybir.dt.np(alloc.dtype)
            out_names.append(name)
            out_avals.append(jax.core.ShapedArray(shape, dtype))
            zero_outs.append(np.zeros(shape, dtype))
    n_params = len(in_names)
    all_in_names = list(in_names) + list(out_names)
    if partition_name is not None:
        all_in_names.append(partition_name)

    def _body(*args):
        operands = list(args)
        if partition_name is not None:
            operands.append(bass2jax.partition_id_tensor())
        outs = bass2jax._bass_exec_p.bind(
            *operands,
            out_avals=tuple(out_avals),
            in_names=tuple(all_in_names),
            out_names=tuple(out_names),
            lowering_input_output_aliases=(),
            sim_require_finite=True,
            sim_require_nnan=True,
            nc=nc,
        )
        return tuple(outs)

    devices = jax.devices()[:NCORES]
    mesh = Mesh(np.asarray(devices), ("core",))
    in_specs = (PartitionSpec("core"),) * (n_params + len(out_names))
    out_specs = (PartitionSpec("core"),) * len(out_names)
    sharded = jax.jit(
        shard_map(
            _body, mesh=mesh, in_specs=in_specs, out_specs=out_specs,
            check_rep=False,
        ),
        keep_unused=True,
    )
    runner = {
        "fn": sharded,
        "in_names": in_names,
        "out_names": out_names,
        "out_avals": out_avals,
        "zero_concat": [
            np.zeros((NCORES * z.shape[0], *z.shape[1:]), z.dtype)
            for z in zero_outs
        ],
        "mesh": mesh,
    }
    _CACHE[key] = runner
    return runner


def _concat_inputs(runner, in_maps):
    return [
        np.concatenate([np.asarray(m[name]) for m in in_maps], axis=0)
        for name in runner["in_names"]
    ]


def run_fast(in_maps, s_dim=S, d_dim=D):
    """Execute via the cached runner; returns list of per-core out dicts."""
    runner = _get_runner(s_dim, d_dim)
    concat_in = _concat_inputs(runner, in_maps)
    out_arrs = runner["fn"](*concat_in, *runner["zero_concat"])
    results = []
    for c in range(NCORES):
        results.append({
            name: np.asarray(out_arrs[i]).reshape(
                NCORES, *runner["out_avals"][i].shape
            )[c]
            for i, name in enumerate(runner["out_names"])
        })
    return results


def timed_run(in_maps, iters=20, s_dim=S, d_dim=D, n_reps=1, variant="full"):
    """Steady-state timing with device-resident inputs. Returns (min_s, all)."""
    import time
    import jax
    from jax.sharding import NamedSharding, PartitionSpec

    runner = _get_runner(s_dim, d_dim, n_reps, variant)
    sh = NamedSharding(runner["mesh"], PartitionSpec("core"))
    dev_in = [jax.device_put(a, sh) for a in _concat_inputs(runner, in_maps)]
    dev_zero = [jax.device_put(z, sh) for z in runner["zero_concat"]]
    jax.block_until_ready(dev_in)
    jax.block_until_ready(dev_zero)
    # warmup (also triggers compile on first use)
    jax.block_until_ready(runner["fn"](*dev_in, *dev_zero))
    times = []
    for _ in range(iters):
        t0 = time.perf_counter()
        jax.block_until_ready(runner["fn"](*dev_in, *dev_zero))
        times.append(time.perf_counter() - t0)
    return min(times), times


def kernel(q, k, v, W, U):
    in_maps = prepare_in_maps(q, k, v, W, U)
    if os.environ.get("BASS_USE_SPMD_HELPER"):
        res = run_spmd(in_maps)
        results = res.results
    else:
        results = run_fast(in_maps)
    out = np.stack([results[b]["out"] for b in range(NCORES)], axis=0)
    return out.astype(np.float32)


def timed_slope(in_maps, iters=30, reps_hi=3, s_dim=S, d_dim=D, variant="full"):
    """True HW kernel time via replication slope: the reps_hi variant runs
    the whole kernel body reps_hi times inside one NEFF. Calls of the two
    variants are interleaved in one loop so slow network drift cancels;
    returns (per_rep_seconds from median pairwise delta, t1_min, thi_min)."""
    import time
    import jax
    from jax.sharding import NamedSharding, PartitionSpec

    runners = {}
    for n in (1, reps_hi):
        r = _get_runner(s_dim, d_dim, n, variant)
        sh = NamedSharding(r["mesh"], PartitionSpec("core"))
        dev_in = [jax.device_put(a, sh) for a in _concat_inputs(r, in_maps)]
        dev_zero = [jax.device_put(z, sh) for z in r["zero_concat"]]
        jax.block_until_ready(dev_in)
        jax.block_until_ready(dev_zero)
        jax.block_until_ready(r["fn"](*dev_in, *dev_zero))  # warm/compile
        runners[n] = (r["fn"], dev_in, dev_zero)

    deltas, t1s, this_ = [], [], []
    for _ in range(iters):
        fn, di, dz = runners[1]
        t0 = time.perf_counter()
        jax.block_until_ready(fn(*di, *dz))
        t1 = time.perf_counter() - t0
        fn, di, dz = runners[reps_hi]
        t0 = time.perf_counter()
        jax.block_until_ready(fn(*di, *dz))
        th = time.perf_counter() - t0
        deltas.append(th - t1)
        t1s.append(t1)
        this_.append(th)
    deltas.sort()
    med = deltas[len(deltas) // 2]
    return med / (reps_hi - 1), min(t1s), min(this_)


# revision 24
# speedup vs baseline: 2.3203x; 2.0550x over previous
"""Trainium2 Bass kernel for nn_Attention_49082886259369.

Computes, per batch b (one batch per NeuronCore, 8 cores data-parallel):
    fac  = tanh(k @ W + q @ U)            [S, D]
    s    = v^T @ fac                      [D, D]
    attn = softmax(s, axis=batch)         <- couples cores
    out  = v @ attn                       [S, D]

v2 design (vs the 3-pass bf16-split v1):
  - All of mm1/mm2 run as single-pass float32r matmuls (PE truncates fp32
    operands to ~fp22, full bf16 rate at moving-dim >= 256). Emulating fp22
    truncation in numpy gives output rel err ~1.0e-3 against the fp64
    reference (gate 2e-2) -- the 3-pass split is unnecessary. PE work drops
    from 10 pass-units (547us floor) to 4 (219us floor).
  - mm2 accumulates over the S-contraction directly in PSUM (16 chained
    matmuls per (di, e-half)) instead of per-mi DVE adds: needs fac for all
    mi resident (64KB/partition) but removes ~137us of DVE work.
  - The dim-0 softmax is done with TWO AllToAlls instead of AR-max+AR-sum:
    each core takes ownership of a 128-wide e-slice, receives s (fp16) for
    all 8 batches, computes max/exp/sum/div locally in fp32, and sends
    attn (bf16) back. A2A moves ~2x fewer wire bytes than AllReduce and
    needs no second reduction. Chunked in 2 di-halves so chunk 0's
    collectives/softmax overlap chunk 1's mm2 and chunk 0's mm3 overlaps
    chunk 1's collectives.
  - mm3 runs in bf16 (attn quantized bf16 on the wire; benign -- attn in
    [0,1], output-proportional error).

Host-side layouts (all DMAs are [128 partitions x contiguous free]):
  kT/qT: [MT, 128, DT, 128] f32  [mi,p,di,sj] = x[mi*128+sj, di*128+p]
  W/U:   [128, DT, D]       f32  [p,di,e]     = W[di*128+p, e]
  vS:    [DT, 128, MT, 128] f32  [di,p,mi,j]  = v[mi*128+p, di*128+j]
  vT:    [MT, 128, DT, 128] bf16 [mi,p,di,tj] = v[mi*128+tj, di*128+p]
"""

import os

import numpy as np
import ml_dtypes

B, S, D = 8, 2048, 1024
NCORES = 8
P = 128
NE = 512  # matmul free-dim tile (one PSUM bank of fp32)
NCH = 2   # di-chunks for collective pipelining

_CACHE: dict = {}


# --------------------------------------------------------------------------
# device kernel builder
# --------------------------------------------------------------------------

def _build_nc(s_dim: int, d_dim: int, n_reps: int = 1, variant: str = "full"):
    import concourse.mybir as mybir
    import concourse.tile as tile
    from concourse import bacc

    F32 = mybir.dt.float32
    F32R = mybir.dt.float32r
    BF16 = mybir.dt.bfloat16
    F16 = mybir.dt.float16
    ACT = mybir.ActivationFunctionType

    MT = s_dim // P          # row tiles of S
    DT = d_dim // P          # row tiles of D
    DC = DT // NCH           # di per chunk
    EC = d_dim // NCORES     # e-slice owned per core in the softmax

    nc = bacc.Bacc("TRN2", target_bir_lowering=False, num_devices=NCORES)

    d_kT = nc.dram_tensor("kT", [MT, P, DT, P], F32R, kind="ExternalInput")
    d_qT = nc.dram_tensor("qT", [MT, P, DT, P], F32R, kind="ExternalInput")
    d_W = nc.dram_tensor("W", [P, DT, d_dim], F32R, kind="ExternalInput")
    d_U = nc.dram_tensor("U", [P, DT, d_dim], F32R, kind="ExternalInput")
    d_vS = nc.dram_tensor("vS", [DT, P, MT, P], F32R, kind="ExternalInput")
    d_vT = nc.dram_tensor("vT", [MT, P, DT, P], BF16, kind="ExternalInput")
    d_out = nc.dram_tensor("out", [s_dim, d_dim], F32, kind="ExternalOutput")

    grp = [list(range(NCORES))]

    def cc_a2a(cin, cout):
        if variant == "nocc":
            nc.gpsimd.dma_start(out=cout[:], in_=cin[:])
        else:
            nc.gpsimd.collective_compute(
                "AllToAll",
                mybir.AluOpType.bypass,
                replica_groups=grp,
                ins=[cin.opt()],
                outs=[cout.opt()],
            )

    with tile.TileContext(nc) as tc:
        with (
            tc.tile_pool(name="dram", bufs=2, space="DRAM") as dram_pool,
        ):
          for _rep in range(n_reps):
            a2a1_out_t = []
            with (
                tc.tile_pool(name="facp", bufs=1) as fac_pool,
                tc.tile_pool(name="s16p", bufs=1) as s16_pool,
                tc.tile_pool(name="vs", bufs=3) as vs_pool,
            ):
                fac = fac_pool.tile([P, MT, d_dim], F32R, tag="fac")
                sA_cm = tc.tile_pool(name="spsA", bufs=1, space="PSUM")
                sA_psum = sA_cm.__enter__()
                NDI_PRE = 2
                sps_pre = {}
                for di in range(NDI_PRE):
                    for ni in range(2):
                        sps_pre[(di, ni)] = sA_psum.tile(
                            [P, NE], F32, tag=f"spsA{di}_{ni}",
                            name=f"spsA{di}_{ni}")
                s16 = s16_pool.tile([P, DT, d_dim], F16, tag="s16")
                # prefetch the first v slices now: their pool doesn't overlap
                # the sweep-A pools, so these DMAs run during sweep A and
                # sweep B can start the moment fac completes
                vs_tiles = {}
                for di in range(2):
                    vs = vs_pool.tile([P, MT, P], F32R, tag="vs",
                                      name=f"vs_pre{di}")
                    nc.sync.dma_start(out=vs, in_=d_vS[di])
                    vs_tiles[di] = vs
                # (di 0/1 feed the interleaved mm2 chains inside sweep A)

                # ---- sweep A: fac = tanh(k@W + q@U), single-pass f32r,
                # with mm2's first two di-chains interleaved (their PSUM
                # banks accumulate across the whole sweep) so chunk 0's s
                # finishes ~18us earlier and the collective chain starts
                # sooner ----
                with (
                    tc.tile_pool(name="wu", bufs=1) as wu_pool,
                    tc.tile_pool(name="kq", bufs=3) as kq_pool,
                    tc.tile_pool(name="fps", bufs=2, space="PSUM") as fac_psum,
                ):
                    # small first k/q tiles ahead of the 4MB W/U loads
                    kt0 = kq_pool.tile([P, DT, P], F32R, tag="kt")
                    qt0 = kq_pool.tile([P, DT, P], F32R, tag="qt")
                    nc.sync.dma_start(out=kt0, in_=d_kT[0])
                    nc.sync.dma_start(out=qt0, in_=d_qT[0])
                    for di in range(2):
                        vs = vs_tiles[di]  # prefetched above
                    w_sb = wu_pool.tile([P, DT, d_dim], F32R, tag="w")
                    u_sb = wu_pool.tile([P, DT, d_dim], F32R, tag="u")
                    # per-di interleaved loads: mi=0's chain starts after just
                    # W[0]/U[0] land and never stalls long enough to lose the
                    # PE clock ramp
                    for di in range(DT):
                        nc.sync.dma_start(out=w_sb[:, di], in_=d_W[:, di])
                        nc.sync.dma_start(out=u_sb[:, di], in_=d_U[:, di])
                    for mi in range(MT):
                        if mi == 0:
                            kt, qt = kt0, qt0
                        else:
                            kt = kq_pool.tile([P, DT, P], F32R, tag="kt")
                            qt = kq_pool.tile([P, DT, P], F32R, tag="qt")
                            nc.sync.dma_start(out=kt, in_=d_kT[mi])
                            nc.sync.dma_start(out=qt, in_=d_qT[mi])
                        fps = [fac_psum.tile([P, NE], F32, tag=f"fps{x}", name=f"fps{x}") for x in range(2)]
                        n_chain = 2 * DT
                        n = 0
                        for di in range(DT):
                            for src, wu in ((kt, w_sb), (qt, u_sb)):
                                for ni in range(2):
                                    nc.tensor.matmul(
                                        fps[ni],
                                        src[:, di, :],
                                        wu[:, di, ni * NE:(ni + 1) * NE],
                                        start=(n == 0),
                                        stop=(n == n_chain - 1),
                                    )
                                n += 1
                        for ni in range(2):
                            nc.scalar.activation(
                                fac[:, mi, ni * NE:(ni + 1) * NE], fps[ni],
                                ACT.Tanh)
                        # interleaved mm2 contributions for di 0..1 of the
                        # PREVIOUS mi (its tanh has completed under this
                        # mi's mm1)
                        if mi > 0:
                            for di in range(NDI_PRE):
                                for ni in range(2):
                                    nc.tensor.matmul(
                                        sps_pre[(di, ni)],
                                        vs_tiles[di][:, mi - 1, :],
                                        fac[:, mi - 1,
                                            ni * NE:(ni + 1) * NE],
                                        start=(mi == 1),
                                        stop=False,
                                    )
                        if mi == MT - 1:
                            for di in range(NDI_PRE):
                                for ni in range(2):
                                    nc.tensor.matmul(
                                        sps_pre[(di, ni)],
                                        vs_tiles[di][:, mi, :],
                                        fac[:, mi, ni * NE:(ni + 1) * NE],
                                        start=False,
                                        stop=True,
                                    )

                # flush + stage the two di-chains accumulated during
                # sweep A, then release their PSUM banks
                cins = [
                        dram_pool.tile([NCORES, P, DC, EC], F16,
                                       tag=f"a2a1_in{ch}", name=f"a2a1_in{ch}")
                        for ch in range(NCH)
                    ]
                couts = [
                    dram_pool.tile([NCORES, P, DC, EC], F16,
                                   tag=f"a2a1_out{ch}",
                                   name=f"a2a1_out{ch}")
                    for ch in range(NCH)
                ]
                for di in range(NDI_PRE):
                    for ni in range(2):
                        nc.vector.tensor_copy(
                            s16[:, di, ni * NE:(ni + 1) * NE],
                            sps_pre[(di, ni)])
                    nc.scalar.dma_start(
                        out=cins[0][:, :, di, :].transpose([1, 0, 2]),
                        in_=s16[:, di, :].rearrange(
                            "p (c e) -> p c e", c=NCORES))
                sA_cm.__exit__(None, None, None)

                # ---- sweep B: s = v^T @ fac, PSUM-accumulated over mi ----
                with (
                    tc.tile_pool(name="sps", bufs=2, space="PSUM") as s_psum,
                ):
                    for ch in range(NCH):
                        for dl in range(DC):
                            di = ch * DC + dl
                            if di < NDI_PRE:
                                continue  # flushed + staged above
                            if True:
                                vs = vs_pool.tile([P, MT, P], F32R, tag="vs",
                                                  name=f"vs{di}")
                                nc.sync.dma_start(out=vs, in_=d_vS[di])
                                sps = [s_psum.tile([P, NE], F32, tag=f"sps{x}", name=f"sps{x}") for x in range(2)]
                                for mi in range(MT):
                                    for ni in range(2):
                                        nc.tensor.matmul(
                                            sps[ni],
                                            vs[:, mi, :],
                                            fac[:, mi, ni * NE:(ni + 1) * NE],
                                            start=(mi == 0),
                                            stop=(mi == MT - 1),
                                        )
                                for ni in range(2):
                                    nc.vector.tensor_copy(
                                        s16[:, di, ni * NE:(ni + 1) * NE],
                                        sps[ni])
                            # stage this di's fp16 s rows immediately so the
                            # chunk's AllToAll is gated only on the last di;
                            # transposed DRAM AP -> one DMA instead of 8
                            nc.scalar.dma_start(
                                out=cins[ch][:, :, dl, :].transpose([1, 0, 2]),
                                in_=s16[:, di, :].rearrange(
                                    "p (c e) -> p c e", c=NCORES))
                        cc_a2a(cins[ch], couts[ch])
                        a2a1_out_t.append(couts[ch])

            # ---- prefetch mm3's v tiles during the collective window ----
            vt_pool_cm = tc.tile_pool(name="vt", bufs=1)
            vt_pool = vt_pool_cm.__enter__()
            vt_all = []
            for ch in range(NCH):
                vt = vt_pool.tile([P, MT, DC, P], BF16, tag=f"vtall{ch}",
                                  name=f"vtall{ch}")
                nc.scalar.dma_start(
                    out=vt,
                    in_=d_vT[:, :, ch * DC:(ch + 1) * DC, :].transpose(
                        [1, 0, 2, 3]))
                vt_all.append(vt)

            # ---- softmax on the owned e-slice + A2A back, per chunk ----
            a2a2_out_t = []
            with tc.tile_pool(name="smp", bufs=1) as sm_pool:
                for ch in range(NCH):
                    r16 = sm_pool.tile([P, NCORES, DC, EC], F16, tag="r16",
                                       name=f"r16_{ch}")
                    nc.sync.dma_start(
                        out=r16,
                        in_=a2a1_out_t[ch][:].transpose([1, 0, 2, 3]))
                    m4 = sm_pool.tile([P, 4, DC, EC], F16, tag="m4")
                    m2 = sm_pool.tile([P, 2, DC, EC], F16, tag="m2")
                    mf = sm_pool.tile([P, DC, EC], F16, tag="mf")
                    for i in range(4):
                        nc.vector.tensor_max(m4[:, i], r16[:, 2 * i],
                                             r16[:, 2 * i + 1])
                    for i in range(2):
                        nc.vector.tensor_max(m2[:, i], m4[:, 2 * i],
                                             m4[:, 2 * i + 1])
                    nc.vector.tensor_max(mf, m2[:, 0], m2[:, 1])
                    e16 = sm_pool.tile([P, NCORES, DC, EC], F16, tag="e16",
                                       name=f"e16_{ch}")
                    for b in range(NCORES):
                        nc.vector.tensor_sub(e16[:, b], r16[:, b], mf)
                    nc.scalar.activation(e16, e16, ACT.Exp)
                    a4 = sm_pool.tile([P, 4, DC, EC], F32, tag="a4")
                    a2 = sm_pool.tile([P, 2, DC, EC], F32, tag="a2")
                    den = sm_pool.tile([P, DC, EC], F32, tag="den")
                    for i in range(4):
                        nc.vector.tensor_add(a4[:, i], e16[:, 2 * i],
                                             e16[:, 2 * i + 1])
                    for i in range(2):
                        nc.vector.tensor_add(a2[:, i], a4[:, 2 * i],
                                             a4[:, 2 * i + 1])
                    nc.vector.tensor_add(den, a2[:, 0], a2[:, 1])
                    nc.vector.reciprocal(den, den)
                    at16 = sm_pool.tile([P, NCORES, DC, EC], BF16, tag="at16",
                                        name=f"at16_{ch}")
                    for b in range(NCORES):
                        nc.vector.tensor_mul(at16[:, b], e16[:, b], den)
                    cin = dram_pool.tile([NCORES, P, DC, EC], BF16,
                                         tag=f"a2a2_in{ch}",
                                         name=f"a2a2_in{ch}")
                    cout = dram_pool.tile([NCORES, P, DC, EC], BF16,
                                          tag=f"a2a2_out{ch}",
                                          name=f"a2a2_out{ch}")
                    nc.scalar.dma_start(
                        out=cin[:].transpose([1, 0, 2, 3]), in_=at16)
                    cc_a2a(cin, cout)
                    a2a2_out_t.append(cout)

            # ---- mm3: out = v @ attn (bf16), chunked over di-halves ----
            with (
                tc.tile_pool(name="attnp", bufs=1) as attn_pool,
                tc.tile_pool(name="ostp", bufs=1) as ost_pool,
                tc.tile_pool(name="ops", bufs=3, space="PSUM") as out_psum,
                tc.tile_pool(name="wrm", bufs=1, space="PSUM") as warm_psum,
            ):
                attn_sb = attn_pool.tile([P, DT, d_dim], BF16, tag="attn")
                ost = [
                    ost_pool.tile([P, d_dim], F32, tag=f"ost{mi}",
                                  name=f"ost{mi}")
                    for mi in range(MT)
                ]
                for ch in range(NCH):
                    dsl = slice(ch * DC, (ch + 1) * DC)
                    if ch > 0:
                        # keep the PE clock ramp alive between mm3 chunks:
                        # ~5us of chained throwaway matmuls bridge the gap
                        # until the last AllToAll lands (HAM re-throttles
                        # after ~3.4us idle)
                        wps = warm_psum.tile([P, NE], F32, tag="warm",
                                             name=f"warm{ch}")
                        for wi in range(24):
                            nc.tensor.matmul(
                                ops[0] if False else wps,
                                vt_all[0][:, 0, 0, :],
                                attn_sb[:, 0, 0:NE],
                                start=(wi == 0),
                                stop=(wi == 23),
                            )
                    nc.sync.dma_start(
                        out=attn_sb[:, dsl, :].rearrange(
                            "p d (c e) -> p d c e", c=NCORES),
                        in_=a2a2_out_t[ch][:].transpose([1, 2, 0, 3]))
                    for mi in range(MT):
                        vt = vt_all[ch]
                        ops = [out_psum.tile([P, NE], F32, tag=f"ops{x}", name=f"ops{x}") for x in range(2)]
                        for dl in range(DC):
                            di = ch * DC + dl
                            for ni in range(2):
                                nc.tensor.matmul(
                                    ops[ni],
                                    vt[:, mi, dl, :],
                                    attn_sb[:, di, ni * NE:(ni + 1) * NE],
                                    start=(dl == 0),
                                    stop=(dl == DC - 1),
                                )
                        for ni in range(2):
                            esl = slice(ni * NE, (ni + 1) * NE)
                            if ch == 0:
                                nc.scalar.activation(ost[mi][:, esl], ops[ni],
                                                     ACT.Copy)
                            else:
                                nc.vector.tensor_add(ost[mi][:, esl],
                                                     ost[mi][:, esl], ops[ni])
                        if ch == NCH - 1:
                            nc.sync.dma_start(
                                out=d_out[mi * P:(mi + 1) * P, :],
                                in_=ost[mi])
            vt_pool_cm.__exit__(None, None, None)
            tc.tile_update_base_wait()

    nc.compile()
    return nc


def _get_nc(s_dim=S, d_dim=D, n_reps=1, variant="full"):
    key = ("nc", s_dim, d_dim, n_reps, variant)
    if key not in _CACHE:
        _CACHE[key] = _build_nc(s_dim, d_dim, n_reps, variant)
    return _CACHE[key]


# --------------------------------------------------------------------------
# host-side packing
# --------------------------------------------------------------------------

def _tileT(x: np.ndarray, s_dim: int, d_dim: int) -> np.ndarray:
    """[S, D] -> [MT, 128, DT, 128] with [mi,p,di,sj] = x[mi*128+sj, di*128+p]."""
    mt, dt = s_dim // P, d_dim // P
    return np.ascontiguousarray(
        x.reshape(mt, P, dt, P).transpose(0, 3, 2, 1)
    )


def prepare_in_maps(q, k, v, W, U, s_dim=S, d_dim=D):
    q = np.asarray(q, dtype=np.float32)
    k = np.asarray(k, dtype=np.float32)
    v = np.asarray(v, dtype=np.float32)
    W = np.asarray(W, dtype=np.float32)
    U = np.asarray(U, dtype=np.float32)

    dt = d_dim // P
    mt = s_dim // P
    W_t = np.ascontiguousarray(W.reshape(dt, P, d_dim).transpose(1, 0, 2))
    U_t = np.ascontiguousarray(U.reshape(dt, P, d_dim).transpose(1, 0, 2))

    in_maps = []
    for b in range(NCORES):
        kT = _tileT(k[b], s_dim, d_dim)
        qT = _tileT(q[b], s_dim, d_dim)
        vT = _tileT(v[b], s_dim, d_dim).astype(ml_dtypes.bfloat16)
        vS = np.ascontiguousarray(
            v[b].reshape(mt, P, dt, P).transpose(2, 1, 0, 3))
        in_maps.append({
            "kT": kT, "qT": qT,
            "W": W_t, "U": U_t,
            "vS": vS, "vT": vT,
        })
    return in_maps


def run_spmd(in_maps, s_dim=S, d_dim=D):
    """One-shot path through the stock bass_utils helper (debug use)."""
    from concourse import bass_utils
    nc = _get_nc(s_dim, d_dim)
    res = bass_utils.run_bass_kernel_spmd(
        nc, in_maps=in_maps, core_ids=list(range(NCORES))
    )
    return res


def _get_runner(s_dim=S, d_dim=D, n_reps=1, variant="full"):
    """Cached sharded-jit runner over the same bass2jax/_bass_exec_p path
    that bass_utils.run_bass_kernel_spmd uses under axon, but built once per
    process (no donation) so repeat calls skip re-trace/re-compile."""
    key = ("runner", s_dim, d_dim, n_reps, variant)
    if key in _CACHE:
        return _CACHE[key]

    import jax
    from jax.sharding import Mesh, PartitionSpec
    from jax.experimental.shard_map import shard_map
    import concourse.mybir as mybir
    from concourse import bass2jax

    nc = _get_nc(s_dim, d_dim, n_reps, variant)
    bass2jax.install_neuronx_cc_hook()

    partition_name = (
        nc.partition_id_tensor.name if nc.partition_id_tensor else None
    )
    in_names, out_names, out_avals, zero_outs = [], [], [], []
    for alloc in nc.m.functions[0].allocations:
        if not isinstance(alloc, mybir.MemoryLocationSet):
            continue
        name = alloc.memorylocations[0].name
        if alloc.kind == "ExternalInput":
            if name != partition_name:
                in_names.append(name)
        elif alloc.kind == "ExternalOutput":
            shape = tuple(alloc.tensor_shape)
            dtype = mybir.dt.np(alloc.dtype)
            out_names.append(name)
            out_avals.append(jax.core.ShapedArray(shape, dtype))
            zero_outs.append(np.zeros(shape, dtype))
    n_params = len(in_names)
    all_in_names = list(in_names) + list(out_names)
    if partition_name is not None:
        all_in_names.append(partition_name)

    def _body(*args):
        operands = list(args)
        if partition_name is not None:
            operands.append(bass2jax.partition_id_tensor())
        outs = bass2jax._bass_exec_p.bind(
            *operands,
            out_avals=tuple(out_avals),
            in_names=tuple(all_in_names),
            out_names=tuple(out_names),
            lowering_input_output_aliases=(),
            sim_require_finite=True,
            sim_require_nnan=True,
            nc=nc,
        )
        return tuple(outs)

    devices = jax.devices()[:NCORES]
    mesh = Mesh(np.asarray(devices), ("core",))
    in_specs = (PartitionSpec("core"),) * (n_params + len(out_names))
    out_specs = (PartitionSpec("core"),) * len(out_names)
    sharded = jax.jit(
        shard_map(
            _body, mesh=mesh, in_specs=in_specs, out_specs=out_specs,
            check_rep=False,
        ),
        keep_unused=True,
    )
    runner = {
        "fn": sharded,
        "in_names": in_names,
        "out_names": out_names,
        "out_avals": out_avals,
        "zero_concat": [
            np.zeros((NCORES * z.shape[0], *z.shape[1:]), z.dtype)
            for z in zero_outs
        ],
        "mesh": mesh,
    }
    _CACHE[key] = runner
    return runner


def _concat_inputs(runner, in_maps):
    return [
        np.concatenate([np.asarray(m[name]) for m in in_maps], axis=0)
        for name in runner["in_names"]
    ]


def run_fast(in_maps, s_dim=S, d_dim=D):
    """Execute via the cached runner; returns list of per-core out dicts."""
    runner = _get_runner(s_dim, d_dim)
    concat_in = _concat_inputs(runner, in_maps)
    out_arrs = runner["fn"](*concat_in, *runner["zero_concat"])
    results = []
    for c in range(NCORES):
        results.append({
            name: np.asarray(out_arrs[i]).reshape(
                NCORES, *runner["out_avals"][i].shape
            )[c]
            for i, name in enumerate(runner["out_names"])
        })
    return results


def timed_run(in_maps, iters=20, s_dim=S, d_dim=D, n_reps=1, variant="full"):
    """Steady-state timing with device-resident inputs. Returns (min_s, all)."""
    import time
    import jax
    from jax.sharding import NamedSharding, PartitionSpec

    runner = _get_runner(s_dim, d_dim, n_reps, variant)
    sh = NamedSharding(runner["mesh"], PartitionSpec("core"))
    dev_in = [jax.device_put(a, sh) for a in _concat_inputs(runner, in_maps)]
    dev_zero = [jax.device_put(z, sh) for z in runner["zero_concat"]]
    jax.block_until_ready(dev_in)
    jax.block_until_ready(dev_zero)
    # warmup (also triggers compile on first use)
    jax.block_until_ready(runner["fn"](*dev_in, *dev_zero))
    times = []
    for _ in range(iters):
        t0 = time.perf_counter()
        jax.block_until_ready(runner["fn"](*dev_in, *dev_zero))
        times.append(time.perf_counter() - t0)
    return min(times), times


def kernel(q, k, v, W, U):
    in_maps = prepare_in_maps(q, k, v, W, U)
    if os.environ.get("BASS_USE_SPMD_HELPER"):
        res = run_spmd(in_maps)
        results = res.results
    else:
        results = run_fast(in_maps)
    out = np.stack([results[b]["out"] for b in range(NCORES)], axis=0)
    return out.astype(np.float32)


def timed_slope(in_maps, iters=30, reps_hi=3, s_dim=S, d_dim=D, variant="full"):
    """True HW kernel time via replication slope: the reps_hi variant runs
    the whole kernel body reps_hi times inside one NEFF. Calls of the two
    variants are interleaved in one loop so slow network drift cancels;
    returns (per_rep_seconds from median pairwise delta, t1_min, thi_min)."""
    import time
    import jax
    from jax.sharding import NamedSharding, PartitionSpec

    runners = {}
    for n in (1, reps_hi):
        r = _get_runner(s_dim, d_dim, n, variant)
        sh = NamedSharding(r["mesh"], PartitionSpec("core"))
        dev_in = [jax.device_put(a, sh) for a in _concat_inputs(r, in_maps)]
        dev_zero = [jax.device_put(z, sh) for z in r["zero_concat"]]
        jax.block_until_ready(dev_in)
        jax.block_until_ready(dev_zero)
        jax.block_until_ready(r["fn"](*dev_in, *dev_zero))  # warm/compile
        runners[n] = (r["fn"], dev_in, dev_zero)

    deltas, t1s, this_ = [], [], []
    for _ in range(iters):
        fn, di, dz = runners[1]
        t0 = time.perf_counter()
        jax.block_until_ready(fn(*di, *dz))
        t1 = time.perf_counter() - t0
        fn, di, dz = runners[reps_hi]
        t0 = time.perf_counter()
        jax.block_until_ready(fn(*di, *dz))
        th = time.perf_counter() - t0
        deltas.append(th - t1)
        t1s.append(t1)
        this_.append(th)
    deltas.sort()
    med = deltas[len(deltas) // 2]
    return med / (reps_hi - 1), min(t1s), min(this_)
